# revision 1
# baseline (speedup 1.0000x reference)
"""BasicMoEBlock kernel for Trainium2 (Bass/Tile), data-parallel over batch on 8 cores.

Computation per sample (matches the reference):
    rw1 = avgpool_experts(sigmoid(mean_hw(x) @ r1_W.T + r1_b))
    out = relu(bn1(conv3x3(x, rw1 @ e1_w)))
    rw2 = avgpool_experts(sigmoid(mean_hw(out) @ r2_W.T + r2_b))
    out = relu(bn2(conv3x3(out, rw2 @ e2_w)) + x)

Mapping:
  - conv3x3 = 18 accumulating PE matmuls (2 ci-chunks x 9 shifts) over a
    zero-padded 34x34 image held in SBUF (bf16), fp32 PSUM accumulation.
  - per-sample expert combination is rw0-factored: w' = W0 + sum_{e>0}
    (rw_e/rw0)*W_e on VectorE (3 multiply-add pairs, bf16); the rw0 factor
    is folded into the BN scale the ACT epilogue applies anyway.
  - x is cast to bf16 on the host (halves its DMA); channel pooling for
    routing rides on the pad-copy's accum_out.
  - routing-weight broadcast across partitions via two tiny PE matmuls.
  - all input DMA shares the sync HWDGE ring in priority order.
"""

import numpy as np
import ml_dtypes

import concourse.bass as bass
import concourse.tile as tile
from concourse import mybir

F32 = mybir.dt.float32
BF16 = mybir.dt.bfloat16
BF16_NP = ml_dtypes.bfloat16

N_CORES = 8
B_LOC = 4          # samples per core
P = 128            # partitions
CI2 = 2            # channel chunks (256 = 2*128)
C = 256
HW = 1024          # 32*32
PADW = 34
PADHW = PADW * PADW
E = 4              # experts
NSH = 9            # 3x3 shifts
EPS = 1e-5
AF = mybir.ActivationFunctionType
OP = mybir.AluOpType


# ---------------------------------------------------------------- kernel build

def _declare_io(nc):
    d = {}

    def din(name, shape, dtype):
        d[name] = nc.dram_tensor(name, shape, dtype, kind="ExternalInput").ap()

    din("x", [B_LOC, C, HW], BF16)
    din("ew1", [P, E, CI2, NSH * C], BF16)
    din("ew2", [P, E, CI2, NSH * C], BF16)
    din("rwt", [P, 2, CI2, C], BF16)    # r{1,2}_W.T, [cin_in, layer, cin_chunk, interm]
    # fp32 blob: rb1[2] rb2[2] inv1[2] shift1[2] inv2[2] shift2[2] mask4[4]
    din("fblob", [P, 16], F32)
    d["out"] = nc.dram_tensor("out", [B_LOC, C, HW], F32, kind="ExternalOutput").ap()
    return d


def _emit(tc, d):
    nc = tc.nc

    with (
        tc.tile_pool(name="const", bufs=1) as const,
        tc.tile_pool(name="wcombp", bufs=3) as wcombp,
        tc.tile_pool(name="xin", bufs=3) as xin,
        tc.tile_pool(name="resp", bufs=3) as resp,
        tc.tile_pool(name="rsb", bufs=4) as rsb,
        tc.tile_pool(name="rps", bufs=2, space="PSUM") as rps,
        tc.tile_pool(name="cps", bufs=3, space="PSUM") as cps,
    ):
        # ---- persistent state
        ew_sb = [const.tile([P, E, CI2, NSH * C], BF16, tag=f"ew{l}", name=f"ew{l}") for l in (0, 1)]
        rwt_all = const.tile([P, 2, CI2, C], BF16, tag="rwtall")
        fblob = const.tile([P, 16], F32, tag="fblob")
        rwt_sb = [rwt_all[:, l] for l in (0, 1)]
        pool_bf = [const.tile([P, B_LOC, CI2], BF16, tag=f"poolbf{l}", name=f"poolbf{l}") for l in (0, 1)]
        rb_sb = [fblob[:, 0:2], fblob[:, 2:4]]
        inv_sb = [fblob[:, 4:6], fblob[:, 8:10]]
        shift_sb = [fblob[:, 6:8], fblob[:, 10:12]]
        mask_sb = fblob[:, 12:16]
        ones_p = const.tile([P, 1], BF16, tag="onesp")
        ones_f = const.tile([1, P], BF16, tag="onesf")
        xpad = const.tile([P, B_LOC, CI2, PADHW], BF16, tag="xpad")
        o1pad = const.tile([P, B_LOC, CI2, PADHW], BF16, tag="o1pad")
        pool_acc = [const.tile([P, B_LOC, CI2], F32, tag=f"pool{l}", name=f"pool{l}") for l in (0, 1)]
        rwbc = [const.tile([P, B_LOC, E], F32, tag=f"rwbc{l}", name=f"rwbc{l}") for l in (0, 1)]
        # rw0-factoring state: per-sample ratios rw_e/rw0 and scaled bn inv
        rat = [const.tile([P, B_LOC, E], F32, tag=f"rat{l}", name=f"rat{l}") for l in (0, 1)]
        invs = [const.tile([P, B_LOC, 2], F32, tag=f"invs{l}", name=f"invs{l}") for l in (0, 1)]

        # tiny DVE constants first so the ACT warm can fire immediately
        nc.vector.memset(ones_p, 1.0)
        nc.vector.memset(ones_f, 1.0)

        # warm the ACT function-table with the sigmoid set before the first
        # real sigmoid lands mid-critical-path (table switch costs ~1.3us);
        # input is ones_p so no DMA gates it
        warm = rsb.tile([P, 1], F32, tag="warm")
        warm_inst = nc.scalar.activation(
            out=warm, in_=ones_p, func=AF.Sigmoid, scale=1.0
        )

        # ---- input DMA. Order matters: the tiny routing/bn tensors go first
        # (they gate sample 0's routing), then x on the sync ring; ew2
        # streams on the gpsimd SWDGE ring in parallel.
        xf_tiles = {}

        def load_x(b):
            for c in range(CI2):
                xf = xin.tile([P, HW], BF16, tag="xf", name=f"xf{b}{c}")
                nc.sync.dma_start(out=xf, in_=d["x"][b, c * P : (c + 1) * P, :])
                xf_tiles[b, c] = xf

        # interleave so sample 0 + the first ci-half of layer-1 weights land
        # first; everything shares the sync HWDGE ring in issue order
        # interleave so sample 0 + the first ci-half of layer-1 weights land
        # first; everything shares the sync HWDGE ring in issue order
        load_x(0)
        nc.sync.dma_start(out=fblob, in_=d["fblob"])
        nc.sync.dma_start(out=rwt_all, in_=d["rwt"])
        for e in range(E):
            nc.sync.dma_start(out=ew_sb[0][:, e, 0], in_=d["ew1"][:, e, 0])
        load_x(1)
        for e in range(E):
            nc.sync.dma_start(out=ew_sb[0][:, e, 1], in_=d["ew1"][:, e, 1])
        load_x(2)
        load_x(3)
        for e in range(E):
            nc.sync.dma_start(out=ew_sb[1][:, e], in_=d["ew2"][:, e])

        # NOTE: trn2's ACTIVATE instruction has a single sync-wait slot, so
        # every nc.scalar.activation below is arranged to have at most ONE
        # cross-engine producer whose semaphore value is not already covered.
        for b in range(B_LOC):
            v = xpad.rearrange("p b c (r q) -> p b c r q", r=PADW)
            nc.vector.memset(v[:, b, :, 0:PADW:33, :], 0.0)
            nc.vector.memset(v[:, b, :, 1:33, 0:PADW:33], 0.0)
        vo = o1pad.rearrange("p b c (r q) -> p b c r q", r=PADW)
        nc.vector.memset(vo[:, :, :, 0:PADW:33, :], 0.0)
        nc.vector.memset(vo[:, :, :, 1:33, 0:PADW:33], 0.0)

        # ---- x: copy into padded layout + channel pooling. Mostly on ACT;
        # sample 0 chunk 1 goes on DVE so both of sample 0's copies run in
        # parallel (shortens the startup critical path).
        for b in range(B_LOC):
            for c in range(CI2):
                dst = xpad[:, b, c].rearrange("p (r q) -> p r q", r=PADW)[:, 1:33, 1:33]
                srcv = xf_tiles[b, c].rearrange("p (r q) -> p r q", r=32)
                if b == 0 and c == 1:
                    nc.vector.tensor_scalar(
                        out=dst, in0=srcv, scalar1=1.0, scalar2=0.0,
                        op0=OP.mult, op1=OP.add,
                        accum_out=pool_acc[0][:, b, c : c + 1],
                    )
                else:
                    ci_ = nc.scalar.activation(
                        out=dst, in_=srcv, func=AF.Copy, scale=1.0,
                        accum_out=pool_acc[0][:, b, c : c + 1],
                    )
                    if b == 0 and c == 0:
                        # force warm first in the ACT stream so the sigmoid
                        # table set is resident before any ACTIVATE runs
                        tile.add_dep_helper(
                            warm_inst.ins, ci_.ins, sync=False,
                            reason="act table preload",
                        )

        def routing(b0, n, l):
            """pool_acc[l][:, b0:b0+n] -> rwbc/rat/invs[l][:, b0:b0+n]."""
            nc.vector.tensor_copy(
                pool_bf[l][:, b0 : b0 + n], pool_acc[l][:, b0 : b0 + n]
            )
            rt_ps = rps.tile([P, CI2, n], F32, tag="rpsA", name="rtps")
            for ic in range(2):
                for cc in range(2):
                    nc.tensor.matmul(
                        rt_ps[:, ic],
                        rwt_sb[l][:, cc, ic * P : (ic + 1) * P],
                        pool_bf[l][:, b0 : b0 + n, cc],
                        start=(cc == 0),
                        stop=(cc == 1),
                    )
            rt2 = rsb.tile([P, CI2, n], F32, tag="rt2", name="rt2")
            for ic in range(2):
                nc.scalar.activation(
                    out=rt2[:, ic],
                    in_=rt_ps[:, ic],
                    func=AF.Sigmoid,
                    bias=rb_sb[l][:, ic : ic + 1],
                    scale=1.0 / HW,
                )
            # masked[p, bb, e] = rt2[p, e>>1, bb] * mask[p, e] (bf16)
            rt_g = bass.AP(
                tensor=rt2.tensor,
                offset=rt2.offset,
                ap=[rt2.ap[0], [1, n], [n, 2], [0, 2]],
            )
            msk_g = bass.AP(
                tensor=mask_sb.tensor,
                offset=mask_sb.offset,
                ap=[mask_sb.ap[0], [0, n], [2, 2], [1, 2]],
            )
            masked = rsb.tile([P, n, E], BF16, tag="masked", name="masked")
            nc.vector.tensor_mul(
                masked.rearrange("p b (h i) -> p b h i", h=2), rt_g, msk_g
            )
            rw1p_ps = rps.tile([1, n * E], F32, tag="rpsA", name="rw1p")
            nc.tensor.matmul(
                rw1p_ps, ones_p, masked.rearrange("p b e -> p (b e)"),
                start=True, stop=True,
            )
            rw1p_sb = rsb.tile([1, n * E], BF16, tag="rw1p", name="rw1psb")
            nc.vector.tensor_copy(rw1p_sb, rw1p_ps)
            # broadcast back to all partitions: [P, n*E]
            rwbc_ps = rps.tile([P, n * E], F32, tag="rpsA", name="rwbcps")
            nc.tensor.matmul(rwbc_ps, ones_f, rw1p_sb, start=True, stop=True)
            nc.vector.tensor_copy(
                rwbc[l][:, b0 : b0 + n].rearrange("p b e -> p (b e)"), rwbc_ps
            )
            # rw0-factoring: r_e = rw_e/rw0 and invs = inv*rw0
            rec = rsb.tile([P, B_LOC, 1], F32, tag="rec", name="rec")
            nc.vector.reciprocal(
                rec[:, b0 : b0 + n], rwbc[l][:, b0 : b0 + n, 0:1]
            )
            rec_g = bass.AP(
                tensor=rec.tensor,
                offset=rec.offset + b0 * rec.ap[1][0],
                ap=[rec.ap[0], [rec.ap[1][0], n], [0, E - 1]],
            )
            nc.vector.tensor_mul(
                rat[l][:, b0 : b0 + n, 1:E],
                rwbc[l][:, b0 : b0 + n, 1:E],
                rec_g,
            )
            for bb in range(b0, b0 + n):
                nc.vector.tensor_scalar(
                    out=invs[l][:, bb],
                    in0=inv_sb[l],
                    scalar1=rwbc[l][:, bb, 0:1],
                    scalar2=None,
                    op0=OP.mult,
                )

        def wcomb_mac(b, l, nchunks=1):
            """combined per-sample conv weights, rw0-factored:
            w' = W0 + sum_{e>=1} (rw_e/rw0) * W_e   (bf16).
            The e=3 multiply rides on the scalar engine except on the
            startup-critical first MAC. Emitted per ci-half (nchunks splits
            each half further for sample 0): conv starts on the first chunk
            early, and ops pace with the chunked weight DMAs."""
            w = wcombp.tile([P, CI2, NSH, C], BF16, tag="wcomb")
            csz = NSH * C // nchunks
            for ci in range(CI2):
                for k in range(nchunks):
                    sl = slice(k * csz, (k + 1) * csz)
                    wv = w[:, ci].rearrange("p s q -> p (s q)")[:, sl]
                    nc.vector.tensor_scalar(
                        out=wv, in0=ew_sb[l][:, 1, ci, sl],
                        scalar1=rat[l][:, b, 1:2], scalar2=None, op0=OP.mult,
                    )
                    nc.vector.tensor_add(wv, wv, ew_sb[l][:, 0, ci, sl])
                    tmp = wcombp.tile([P, NSH * C], BF16, tag="wtmp", name="wtmp")
                    nc.vector.tensor_scalar(
                        out=tmp[:, sl], in0=ew_sb[l][:, 2, ci, sl],
                        scalar1=rat[l][:, b, 2:3], scalar2=None, op0=OP.mult,
                    )
                    nc.vector.tensor_add(wv, wv, tmp[:, sl])
                    tmp2 = wcombp.tile([P, NSH * C], BF16, tag="wtmp2", name="wtmp2")
                    if not (b == 0 and l == 0):
                        # offload one multiply per half to the scalar engine
                        # (except the startup-critical first MAC: the ACT
                        # queue latency would gate the first conv)
                        nc.scalar.activation(
                            out=tmp2[:, sl], in_=ew_sb[l][:, 3, ci, sl],
                            func=AF.Copy, scale=rat[l][:, b, 3:4],
                        )
                    else:
                        nc.vector.tensor_scalar(
                            out=tmp2[:, sl], in0=ew_sb[l][:, 3, ci, sl],
                            scalar1=rat[l][:, b, 3:4], scalar2=None, op0=OP.mult,
                        )
                    nc.vector.tensor_add(wv, wv, tmp2[:, sl])
            return w

        def conv(b, w, srcpad, hh_outer=False):
            """3x3 same conv: 18 accumulating matmuls per (co, h-half). Returns
            two [P, 1024] fp32 psum tiles (co chunks). hh_outer finishes each
            h-half's accumulation group before starting the next, letting the
            epilogue overlap the tail of the conv."""
            psums = []
            for co in range(2):
                ps = cps.tile([P, HW], F32, tag="convps")
                hh_rng = range(2) if hh_outer else [None]
                for hh0 in hh_rng:
                    for ci in range(2):
                        src34 = srcpad[:, b, ci].rearrange("p (r q) -> p r q", r=PADW)
                        for s in range(NSH):
                            ky, kx = divmod(s, 3)
                            lhsT = w[:, ci, s, co * P : (co + 1) * P]
                            for hh in ([hh0] if hh_outer else range(2)):
                                rhs = src34[:, ky + hh * 16 : ky + hh * 16 + 16, kx : kx + 32]
                                nc.tensor.matmul(
                                    ps[:, hh * 512 : (hh + 1) * 512],
                                    lhsT,
                                    rhs,
                                    start=(ci == 0 and s == 0),
                                    stop=(ci == 1 and s == NSH - 1),
                                )
                psums.append(ps)
            return psums

        def bn1_relu(b, psums):
            for co in range(2):
                dst = o1pad[:, b, co].rearrange("p (r q) -> p r q", r=PADW)[:, 1:33, 1:33]
                nc.scalar.activation(
                    out=dst,
                    in_=psums[co].rearrange("p (r q) -> p r q", r=32),
                    func=AF.Relu,
                    bias=shift_sb[0][:, co : co + 1],
                    scale=invs[0][:, b, co : co + 1],
                    accum_out=pool_acc[1][:, b, co : co + 1],
                )

        def bn2_res(b, psums, split=False):
            halves = range(2) if split else [None]
            for co in range(2):
                res = resp.tile([P, HW], F32, tag="res")
                for hh in halves:
                    sl = slice(None) if hh is None else slice(hh * 512, (hh + 1) * 512)
                    rows = 32 if hh is None else 16
                    r0 = 0 if hh is None else hh * 16
                    resv = res[:, sl].rearrange("p (r q) -> p r q", r=rows)
                    xv = xpad[:, b, co].rearrange("p (r q) -> p r q", r=PADW)[
                        :, 1 + r0 : 1 + r0 + rows, 1:33]
                    psv = psums[co][:, sl].rearrange("p (r q) -> p r q", r=rows)
                    # res = psum*(inv2*rw0) + x ; res = max(res + shift2, 0)
                    nc.vector.scalar_tensor_tensor(
                        out=resv, in0=psv, scalar=invs[1][:, b, co : co + 1], in1=xv,
                        op0=OP.mult, op1=OP.add,
                    )
                    nc.scalar.activation(
                        out=res[:, sl], in_=res[:, sl], func=AF.Relu,
                        bias=shift_sb[1][:, co : co + 1], scale=1.0,
                    )
                    nc.sync.dma_start(
                        out=d["out"][b, co * P : (co + 1) * P, sl], in_=res[:, sl]
                    )

        # ---- main pipeline
        w1 = []
        for b in range(B_LOC):
            routing(b, 1, 0)
            w1.append(wcomb_mac(b, 0))
        w2 = {}
        for b in range(B_LOC):
            ps = conv(b, w1[b], xpad)
            bn1_relu(b, ps)
            if b % 2 == 1:
                # batched pair: halves the tiny routing-matmul count; the
                # layer-2 weight MACs have >15us of slack at this point
                routing(b - 1, 2, 1)
                w2[b - 1] = wcomb_mac(b - 1, 1)
                w2[b] = wcomb_mac(b, 1)
        for b in range(B_LOC):
            last = b == B_LOC - 1
            ps = conv(b, w2[b], o1pad, hh_outer=last)
            bn2_res(b, ps, split=last)


_NC_CACHE = {}


def _build_nc():
    if "nc" not in _NC_CACHE:
        import concourse.bacc as bacc

        # Bacc (not raw Bass): its compile() runs split_sync_waits, which
        # legalizes multi-wait instructions for TRN2's 1-wait-per-inst ISA.
        nc = bacc.Bacc("TRN2", target_bir_lowering=False)
        d = _declare_io(nc)
        with tile.TileContext(nc) as tc:
            _emit(tc, d)
        nc.compile()
        _NC_CACHE["nc"] = nc
    return _NC_CACHE["nc"]


# ---------------------------------------------------------------- host prep

def _prep_ew(e_w):
    # [4, 589824] -> [ci_in(128), e, ci_chunk, (ky kx co)]  bf16
    w = np.asarray(e_w, np.float32).reshape(E, C, CI2, P, 3, 3)
    w = w.transpose(3, 0, 2, 4, 5, 1)  # ci_in, e, ci_chunk, ky, kx, co
    return np.ascontiguousarray(w.reshape(P, E, CI2, NSH * C)).astype(BF16_NP)


def _prep_rwt(rW):
    # [interm, cin] -> transpose -> [cin_in(128), cin_chunk, interm]
    t = np.asarray(rW, np.float32).T.reshape(CI2, P, C).transpose(1, 0, 2)
    return np.ascontiguousarray(t).astype(BF16_NP)


def _prep_vec(v):
    return np.ascontiguousarray(np.asarray(v, np.float32).reshape(CI2, P).T)


def _fold_bn(g, b, m, v):
    inv = np.asarray(g, np.float32) / np.sqrt(np.asarray(v, np.float32) + EPS)
    shift = np.asarray(b, np.float32) - np.asarray(m, np.float32) * inv
    return _prep_vec(inv), _prep_vec(shift)


def _mask4():
    m = np.zeros((P, E), np.float32)
    for e in range(E):
        lo = 64 * (e % 2)
        m[lo : lo + 64, e] = 1.0 / 64.0
    return m


def _prep_inputs(inputs):
    inv1, shift1 = _fold_bn(inputs["bn1_gamma"], inputs["bn1_beta"],
                            inputs["bn1_mean"], inputs["bn1_var"])
    inv2, shift2 = _fold_bn(inputs["bn2_gamma"], inputs["bn2_beta"],
                            inputs["bn2_mean"], inputs["bn2_var"])
    fblob = np.concatenate(
        [_prep_vec(inputs["r1_b"]), _prep_vec(inputs["r2_b"]),
         inv1, shift1, inv2, shift2, _mask4()], axis=1
    )
    rwt = np.stack([_prep_rwt(inputs["r1_W"]), _prep_rwt(inputs["r2_W"])], axis=1)
    shared = {
        "ew1": _prep_ew(inputs["e1_w"]),
        "ew2": _prep_ew(inputs["e2_w"]),
        "rwt": np.ascontiguousarray(rwt),
        "fblob": np.ascontiguousarray(fblob),
    }
    x8 = np.ascontiguousarray(
        np.asarray(inputs["x"], np.float32).reshape(N_CORES, B_LOC, C, HW)
    ).astype(BF16_NP)
    return shared, x8


def _run(inputs, trace=False):
    from concourse.bass_utils import run_bass_kernel_spmd

    nc = _build_nc()
    shared, x8 = _prep_inputs(inputs)
    in_maps = [{"x": x8[c], **shared} for c in range(N_CORES)]
    r = run_bass_kernel_spmd(nc, in_maps, list(range(N_CORES)), trace=trace)
    out = np.stack([np.asarray(r.results[c]["out"]) for c in range(N_CORES)])
    return out.reshape(32, C, 32, 32).astype(np.float32), r


def kernel(**inputs):
    out, _ = _run(inputs, trace=False)
    return out


def _install_ntff_shim():
    """The image's antenv package lacks axon_hooks; recreate it and register
    the ctypes NTFF profile hook the way trn_boot would have."""
    import sys
    import types

    if "antenv.axon_hooks" in sys.modules:
        return
    mod = types.ModuleType("antenv.axon_hooks")
    state = {"hook": None}
    mod.set_axon_ntff_profile_hook = lambda h: state.update(hook=h)
    mod.get_axon_ntff_profile_hook = lambda: state["hook"]
    sys.modules["antenv.axon_hooks"] = mod
    import antenv

    antenv.axon_hooks = mod
    try:
        from trn_agent_boot.trn_boot import _ntff_profile_via_ctypes

        mod.set_axon_ntff_profile_hook(
            _ntff_profile_via_ctypes("/opt/axon/libaxon_pjrt.so")
        )
    except Exception as e:  # degrade to no tracing
        print(f"ntff shim failed: {e}")


def run_traced(inputs):
    _install_ntff_shim()
    out, r = _run(inputs, trace=True)
    return out, r


def run_sim(inputs):
    """CoreSim of core 0's shard. Returns [B_LOC, C, 32, 32]."""
    from concourse.bass_interp import CoreSim

    nc = _build_nc()
    shared, x8 = _prep_inputs(inputs)
    sim = CoreSim(nc)
    for k, v in {"x": x8[0], **shared}.items():
        sim.tensor(k)[:] = v
    sim.simulate()
    return np.asarray(sim.tensor("out")).reshape(B_LOC, C, 32, 32).copy()

